# revision 2
# baseline (speedup 1.0000x reference)
"""Inclusive prefix-sum (Blelloch scan, additive) along L for X_in (8, 4096, 64, 16) f32.

Strategy: shard batch B=8 across the 8 NeuronCores (one batch per core). Per core the
problem is a cumsum along L=4096 of a (L, F=1024) f32 matrix.

Per-core kernel ("transposed-output matmul scan"):
  - View x as (L, F). For each 128-row L-block i and 128-wide feature group g, one
    fp32 matmul with the *data* as the stationary operand and an upper-triangular
    ones matrix U as the moving operand:
        psum[f_local, l_local] = sum_{k <= l_local} x[128*i + k, 128*g + f_local]
    i.e. the within-block inclusive scan, with F on partitions (transposed).
  - The inter-block carry is then a per-partition scalar, added during the
    PSUM -> SBUF copy (vector tensor_scalar_add / scalar activation bias). Carry for
    block i is column (128*i - 1) of the already-written staged output.
  - Output is written transposed, y (F, L); numpy un-transposes when unsharding.

All DMA transfers are dense row-major slices (4 KiB+ per partition row).
"""

import numpy as np

B, L, D, N = 8, 4096, 64, 16
F = D * N            # 1024 features per batch
NCORES = 8
LBLK = 128           # L positions per matmul block
NGROUP = F // 128    # 8 feature groups
NBLK = L // LBLK     # 32 L-blocks
SPAN = 1024          # L columns per staged output tile
BLKS_PER_SPAN = SPAN // LBLK
NSPAN = L // SPAN

_CACHE = {}


def _build_nc():
    import concourse.bacc as bacc
    import concourse.mybir as mybir
    from concourse.tile import TileContext

    f32 = mybir.dt.float32
    nc = bacc.Bacc(
        "TRN2", target_bir_lowering=False, debug=False, num_devices=NCORES
    )
    x = nc.dram_tensor("x", (L, F), f32, kind="ExternalInput")
    u = nc.dram_tensor("u", (LBLK, LBLK), f32, kind="ExternalInput")
    y = nc.dram_tensor("y", (F, L), f32, kind="ExternalOutput")

    with TileContext(nc) as tc:
        with (
            tc.tile_pool(name="const", bufs=1) as cpool,
            tc.tile_pool(name="xin", bufs=4) as xpool,
            tc.tile_pool(name="stage", bufs=2) as spool,
            tc.tile_pool(name="psum", bufs=8, space="PSUM") as ppool,
        ):
            ut = cpool.tile([LBLK, LBLK], f32)
            nc.sync.dma_start(out=ut[:], in_=u[:, :])

            staged = [None] * NGROUP
            prev_staged = [None] * NGROUP
            for i in range(NBLK):
                s, ib = divmod(i, BLKS_PER_SPAN)
                xt = xpool.tile([LBLK, F], f32)
                nc.sync.dma_start(out=xt[:], in_=x[i * LBLK : (i + 1) * LBLK, :])
                for g in range(NGROUP):
                    if ib == 0:
                        prev_staged[g] = staged[g]
                        staged[g] = spool.tile(
                            [128, SPAN], f32, tag=f"st{g}", name=f"st{g}_{s}"
                        )
                    ps = ppool.tile([128, LBLK], f32)
                    # psum = x_block_g.T scanned along L (F on partitions)
                    nc.tensor.matmul(
                        ps[:],
                        xt[:, g * 128 : (g + 1) * 128],
                        ut[:],
                        start=True,
                        stop=True,
                    )
                    dst = staged[g][:, ib * LBLK : (ib + 1) * LBLK]
                    on_dve = g < NGROUP // 2
                    if i == 0:
                        if on_dve:
                            nc.vector.tensor_copy(out=dst, in_=ps[:])
                        else:
                            nc.scalar.copy(out=dst, in_=ps[:])
                    else:
                        carry = (
                            staged[g][:, ib * LBLK - 1 : ib * LBLK]
                            if ib > 0
                            else prev_staged[g][:, SPAN - 1 : SPAN]
                        )
                        if on_dve:
                            nc.vector.tensor_scalar_add(
                                out=dst, in0=ps[:], scalar1=carry
                            )
                        else:
                            nc.scalar.add(out=dst, in_=ps[:], add=carry)
                    if ib == BLKS_PER_SPAN - 1:
                        nc.sync.dma_start(
                            out=y[g * 128 : (g + 1) * 128, s * SPAN : (s + 1) * SPAN],
                            in_=staged[g][:],
                        )
    nc.compile()
    return nc


def _get_nc():
    if "nc" not in _CACHE:
        _CACHE["nc"] = _build_nc()
    return _CACHE["nc"]


def _make_in_maps(X_in):
    xs = np.ascontiguousarray(np.asarray(X_in, dtype=np.float32)).reshape(B, L, F)
    umat = np.triu(np.ones((LBLK, LBLK), dtype=np.float32))
    return [{"x": xs[b], "u": umat} for b in range(B)]


def _unshard(per_core_outs):
    out = np.empty((B, L, D, N), dtype=np.float32)
    for b in range(B):
        out[b] = per_core_outs[b]["y"].T.reshape(L, D, N)
    return out


def kernel(X_in):
    from concourse.bass_utils import run_bass_kernel_spmd

    nc = _get_nc()
    res = run_bass_kernel_spmd(nc, _make_in_maps(X_in), core_ids=list(range(NCORES)))
    return _unshard(res.results)


# revision 5
# speedup vs baseline: 221.0064x; 221.0064x over previous
"""Inclusive prefix-sum (Blelloch scan, additive) along L for X_in (8, 4096, 64, 16) f32.

Sharding: batch B=8 across the 8 NeuronCores (one batch per core; no communication).
Per core the problem is a cumsum along L=4096 of a (L, F=1024) f32 matrix.

Per-core kernel ("transposed-output matmul scan"):
  - For each 128-row L-block i and 128-wide feature group g, one fp32 matmul with the
    *data* as the stationary operand and an upper-triangular ones matrix U as the
    moving operand:
        psum[f_local, l_local] = sum_{k <= l_local} x[128*i + k, 128*g + f_local]
    i.e. the within-block inclusive scan, transposed so F is on partitions.
  - The inter-block carry is then a per-partition scalar, fused into the PSUM->SBUF
    copy (vector tensor_scalar_add for groups 0-3, scalar-engine activation bias for
    groups 4-7). Carry for block i is the last already-written column of the staged
    output; block 0 uses a zeros column so every block runs the identical op
    (avoids ACT activation-table switches). The PE has no serial chains; the 8
    carry chains run on DVE/ACT and overlap with DMA.
  - Input DMA: 1 MiB chunks (two L-blocks per transfer) on the sync HWDGE ring.
    Output staged in (128, 2048) tiles -> 1 MiB out-DMAs, also on sync.
    (gpsimd/scalar DMA rings measured slightly faster but crash intermittently
    with concurrent DVE activity — sync-only is stable.)
  - Output is written transposed, y (F, L); numpy un-transposes when unsharding.

Measured (For_i loop-diff on HW, 8 cores concurrent): ~98 us/iteration true exec,
vs ~93 us pure-DMA floor (358 GB/s HBM-per-core limit on 32 MiB of traffic).
"""

import numpy as np

B, L, D, N = 8, 4096, 64, 16
F = D * N            # 1024 features per batch
NCORES = 8
LBLK = 128           # L positions per matmul block
NGROUP = F // 128    # 8 feature groups
NBLK = L // LBLK     # 32 L-blocks
SPAN = 2048          # L columns per staged output tile (1 MiB out-DMAs)
BLKS_PER_SPAN = SPAN // LBLK
XIN_BUFS = 6

_CACHE = {}


def _build_nc(loop_nrep=None):
    """Build the Bass program. loop_nrep wraps the body in a device-side For_i —
    used only by test.py for timing (the graded path uses loop_nrep=None)."""
    from contextlib import nullcontext

    import concourse.bacc as bacc
    import concourse.mybir as mybir
    from concourse.tile import TileContext

    f32 = mybir.dt.float32
    nc = bacc.Bacc(
        "TRN2", target_bir_lowering=False, debug=False, num_devices=NCORES
    )
    x = nc.dram_tensor("x", (L, F), f32, kind="ExternalInput")
    u = nc.dram_tensor("u", (LBLK, LBLK), f32, kind="ExternalInput")
    y = nc.dram_tensor("y", (F, L), f32, kind="ExternalOutput")

    with TileContext(nc) as tc:
        with (
            tc.tile_pool(name="const", bufs=1) as cpool,
            tc.tile_pool(name="xin", bufs=XIN_BUFS) as xpool,
            tc.tile_pool(name="stage", bufs=2) as spool,
            tc.tile_pool(name="psum", bufs=8, space="PSUM") as ppool,
        ):
            ut = cpool.tile([LBLK, LBLK], f32)
            nc.sync.dma_start(out=ut[:], in_=u[:, :])
            zt = cpool.tile([128, 1], f32)
            nc.vector.memset(zt[:], 0.0)

            loop_cm = tc.For_i(0, loop_nrep, 1) if loop_nrep else nullcontext()
            loop_cm.__enter__()
            staged = [None] * NGROUP
            prev_staged = [None] * NGROUP
            for ii in range(NBLK // 2):  # 1 MiB input chunks: 2 L-blocks each
                xt = xpool.tile([128, 2 * F], f32, tag="xt", name=f"xt_{ii}")
                nc.sync.dma_start(
                    out=xt[:],
                    in_=x[ii * 256 : (ii + 1) * 256, :].rearrange(
                        "(t p) f -> p t f", p=128
                    ),
                )
                for t in range(2):
                    i = 2 * ii + t
                    s, ib = divmod(i, BLKS_PER_SPAN)
                    for g in range(NGROUP):
                        if ib == 0:
                            prev_staged[g] = staged[g]
                            staged[g] = spool.tile(
                                [128, SPAN], f32, tag=f"st{g}", name=f"st{g}_{s}"
                            )
                        ps = ppool.tile([128, LBLK], f32, tag="ps", name=f"ps_{i}_{g}")
                        nc.tensor.matmul(
                            ps[:],
                            xt[:, t * F + g * 128 : t * F + (g + 1) * 128],
                            ut[:],
                            start=True,
                            stop=True,
                        )
                        dst = staged[g][:, ib * LBLK : (ib + 1) * LBLK]
                        if i == 0:
                            carry = zt[:]
                        elif ib > 0:
                            carry = staged[g][:, ib * LBLK - 1 : ib * LBLK]
                        else:
                            carry = prev_staged[g][:, SPAN - 1 : SPAN]
                        if g < NGROUP // 2:
                            nc.vector.tensor_scalar_add(
                                out=dst, in0=ps[:], scalar1=carry
                            )
                        else:
                            nc.scalar.add(out=dst, in_=ps[:], add=carry)
                        if ib == BLKS_PER_SPAN - 1:
                            nc.sync.dma_start(
                                out=y[
                                    g * 128 : (g + 1) * 128, s * SPAN : (s + 1) * SPAN
                                ],
                                in_=staged[g][:],
                            )
            loop_cm.__exit__(None, None, None)
    nc.compile()
    return nc


def _get_nc():
    if "nc" not in _CACHE:
        _CACHE["nc"] = _build_nc()
    return _CACHE["nc"]


def _make_in_maps(X_in):
    xs = np.ascontiguousarray(np.asarray(X_in, dtype=np.float32)).reshape(B, L, F)
    umat = np.triu(np.ones((LBLK, LBLK), dtype=np.float32))
    return [{"x": xs[b], "u": umat} for b in range(B)]


def _unshard(per_core_outs):
    out = np.empty((B, L, D, N), dtype=np.float32)
    for b in range(B):
        out[b] = per_core_outs[b]["y"].T.reshape(L, D, N)
    return out


def kernel(X_in):
    from concourse.bass_utils import run_bass_kernel_spmd

    nc = _get_nc()
    res = run_bass_kernel_spmd(nc, _make_in_maps(X_in), core_ids=list(range(NCORES)))
    return _unshard(res.results)
